# revision 8
# baseline (speedup 1.0000x reference)
"""AttentionBlock (GroupNorm+SiLU -> qkv -> 8-head attn -> proj -> residual)
on 8 TRN2 NeuronCores, head-parallel.

Key structure: the torch-faithful reshape q.transpose(1,2).reshape(B*NH,N,d)
makes "head" h = spatial positions n in [512h, 512h+512) -- attention is
block-diagonal over spatial blocks, so each core independently computes the
full pipeline for its block of 512 spatial positions and emits the final
output columns out[:, 512h:512h+512].

Sequence-axis permutation freedom (attention is equivariant under a common
permutation of Q/K/V rows) lets us use t = chunk*512 + n' ordering
(chunk = c//64, n' = spatial), which makes every layout a cheap copy.

Perf structure vs v1:
- GroupNorm stats split across DVE (bn_stats, chunks 0-1) and ACT
  (Square/Copy with accum_out, chunks 2-3), fed by a bf16 copy of x
  (half the DMA) while the core's own f32 block loads in parallel.
- S-matmuls (K=64) run 2x via PE row tiling: tiles (0,0) and (64,0)
  process even/odd j-blocks concurrently.  K^T and Q live duplicated
  on both partition halves (built by two strided mirror DMAs).
- Softmax skips max-subtraction (scores*scale within [-0.76, 0.86]).
"""

import sys

if "/opt/trn_rl_repo" not in sys.path:
    sys.path.append("/opt/trn_rl_repo")  # fallback; the axon-site copy wins

import numpy as np

import concourse.bacc as bacc
import concourse.tile as tile
from concourse import mybir
from concourse.bass_utils import run_bass_kernel_spmd

F32 = mybir.dt.float32
F32R = mybir.dt.float32r
BF16 = mybir.dt.bfloat16
AF = mybir.ActivationFunctionType

CH = 512          # channels
N = 4096          # spatial positions (64*64)
NB = 512          # spatial block per core
NCORES = 8
G = 32            # groups
GS = 16           # channels per group
EPS = 1e-5
SCALE = 0.125     # d ** -0.5, d = 64

TILE_S = True     # row-tile the S matmuls (2x concurrency)


def _build():
    nc = bacc.Bacc(None, target_bir_lowering=False)

    xfb = nc.declare_dram_parameter("xfb", [CH, N], BF16, isOutput=False)
    xblk = nc.declare_dram_parameter("xblk", [CH, NB], F32, isOutput=False)
    qkvwT = nc.declare_dram_parameter("qkvwT", [CH, 3 * CH], F32R, isOutput=False)
    qb = nc.declare_dram_parameter("qb", [128, 12], F32, isOutput=False)
    pwT = nc.declare_dram_parameter("pwT", [64, 8 * CH], F32R, isOutput=False)
    pb = nc.declare_dram_parameter("pb", [128, 4], F32, isOutput=False)
    nw = nc.declare_dram_parameter("nw", [128, 4], F32, isOutput=False)
    nbias = nc.declare_dram_parameter("nbias", [128, 4], F32, isOutput=False)
    identb = nc.declare_dram_parameter("identb", [128, 128], BF16, isOutput=False)
    ones64 = nc.declare_dram_parameter("ones64", [1, 64], F32R, isOutput=False)
    sel8 = nc.declare_dram_parameter("sel8", [128, 8], F32, isOutput=False)
    selT = nc.declare_dram_parameter("selT", [8, 128], F32, isOutput=False)
    out = nc.declare_dram_parameter("out", [CH, NB], F32, isOutput=True)

    with tile.TileContext(nc) as tc:
        _emit(nc, tc, locals())
    nc.finalize()
    return nc


def _emit(nc, tc, P):
    from contextlib import ExitStack

    xfb, xblk, qkvwT, qb, pwT, pb = (P[k] for k in
        ("xfb", "xblk", "qkvwT", "qb", "pwT", "pb"))
    nw, nbias, identb, ones64, sel8, selT, out = (P[k] for k in
        ("nw", "nbias", "identb", "ones64", "sel8", "selT", "out"))

    with ExitStack() as es:
        # ---------- persistent pools ----------
        persist = es.enter_context(tc.tile_pool(name="persist", bufs=1))
        consts = es.enter_context(tc.tile_pool(name="consts", bufs=1))

        xblk_sb = persist.tile([128, 4 * NB], F32)          # [p, t*512+n']
        pwT_sb = persist.tile([64, 8 * CH], F32R)           # [p, chunk*512+o]
        QT = persist.tile([128, N], F32R)                   # both halves hold all chunks
        KT = persist.tile([128, N], F32R)
        Vp = persist.tile([128, 32 * 65], BF16)             # [V_j | ones]
        ONorm = persist.tile([64, N], F32R)

        qb_sb = consts.tile([128, 12], F32)
        pb_sb = consts.tile([128, 4], F32)
        nw_sb = consts.tile([128, 4], F32)
        nb_sb = consts.tile([128, 4], F32)
        id_sb = consts.tile([128, 128], BF16)
        ones64_sb = consts.tile([1, 64], F32R)
        sel8_sb = consts.tile([128, 8], F32)
        selT_sb = consts.tile([8, 128], F32)
        eps_sb = consts.tile([128, 1], F32)
        A_sb = consts.tile([128, 4], F32)
        B_sb = consts.tile([128, 4], F32)

        # ---------- phase B: loads + GroupNorm stats (DVE || ACT) ----------
        with ExitStack() as es_b, ExitStack() as es_cd:
            pools = es_b.enter_context(tc.tile_pool(name="pools", bufs=4))
            poolbig = es_b.enter_context(tc.tile_pool(name="poolbig", bufs=1))
            psA = es_b.enter_context(tc.tile_pool(name="psA", bufs=1, space="PSUM"))

            # stats source: bf16 full x; sync queue gets these first
            xfb_sb = poolbig.tile([128, 4 * N], BF16, tag="xfb")
            for t in range(4):
                nc.sync.dma_start(out=xfb_sb[:, t * N:(t + 1) * N],
                                  in_=xfb[t * 128:(t + 1) * 128, :])
            # own block f32 (residual + silu input) on the gpsimd queue
            for t in range(4):
                nc.gpsimd.dma_start(out=xblk_sb[:, t * NB:(t + 1) * NB],
                                    in_=xblk[t * 128:(t + 1) * 128, :])
            # weights on the scalar queue
            qkvw_sb = es_cd.enter_context(
                tc.tile_pool(name="poolq", bufs=1)).tile([128, 4 * 1536], F32R)
            for kt in range(4):
                nc.scalar.dma_start(out=qkvw_sb[:, kt * 1536:(kt + 1) * 1536],
                                    in_=qkvwT[kt * 128:(kt + 1) * 128, :])
            nc.scalar.dma_start(out=pwT_sb[:], in_=pwT[:])
            # consts on the gpsimd queue
            nc.gpsimd.dma_start(out=sel8_sb[:], in_=sel8[:])
            nc.gpsimd.dma_start(out=selT_sb[:], in_=selT[:])
            nc.gpsimd.dma_start(out=nw_sb[:], in_=nw[:])
            nc.gpsimd.dma_start(out=nb_sb[:], in_=nbias[:])
            nc.gpsimd.dma_start(out=qb_sb[:], in_=qb[:])
            nc.gpsimd.dma_start(out=pb_sb[:], in_=pb[:])
            nc.gpsimd.dma_start(out=id_sb[:], in_=identb[:])
            nc.gpsimd.dma_start(out=ones64_sb[:], in_=ones64[:])
            nc.vector.memset(eps_sb[:], EPS)

            chs = pools.tile([128, 8], F32, tag="chs")      # [mean_t, ex2_t]*4
            # chunks 0-1: DVE bn_stats
            for t in (0, 1):
                st = pools.tile([128, 8, 6], F32, tag="st")
                for k in range(8):
                    nc.vector.bn_stats(out=st[:, k, :],
                                       in_=xfb_sb[:, t * N + k * 512:
                                                  t * N + (k + 1) * 512])
                mv = pools.tile([128, 2], F32, tag="mv")
                nc.vector.bn_aggr(out=mv[:], in_=st[:])
                nc.vector.tensor_copy(chs[:, 2 * t:2 * t + 1], mv[:, 0:1])
                msq = pools.tile([128, 1], F32, tag="msq")
                nc.vector.tensor_tensor(out=msq[:], in0=mv[:, 0:1], in1=mv[:, 0:1],
                                        op=mybir.AluOpType.mult)
                nc.vector.tensor_tensor(out=chs[:, 2 * t + 1:2 * t + 2],
                                        in0=msq[:], in1=mv[:, 1:2],
                                        op=mybir.AluOpType.add)
            # chunks 2-3: ACT accumulate (sum x, sum x^2); square/copy are in
            # every act table set so this costs no extra table loads
            sq = poolbig.tile([128, N], BF16, tag="sq")
            for t in (2, 3):
                a2 = pools.tile([128, 1], F32, tag="a2")
                a1 = pools.tile([128, 1], F32, tag="a1")
                nc.scalar.activation(out=sq[:], in_=xfb_sb[:, t * N:(t + 1) * N],
                                     func=AF.Square, accum_out=a2[:])
                nc.scalar.activation(out=sq[:], in_=xfb_sb[:, t * N:(t + 1) * N],
                                     func=AF.Copy, accum_out=a1[:])
                nc.vector.tensor_scalar_mul(chs[:, 2 * t:2 * t + 1], a1[:],
                                            1.0 / N)
                nc.vector.tensor_scalar_mul(chs[:, 2 * t + 1:2 * t + 2], a2[:],
                                            1.0 / N)

            # group fold: per-channel -> per-group (sel8 = 1/16), then back
            gp = psA.tile([8, 8], F32, tag="gp")
            for t in range(4):
                nc.tensor.matmul(gp[:, 2 * t:2 * t + 2], lhsT=sel8_sb[:],
                                 rhs=chs[:, 2 * t:2 * t + 2], start=True, stop=True)
            gp_sb = pools.tile([8, 8], F32, tag="gpsb")
            nc.vector.tensor_scalar_mul(gp_sb[:], gp[:], float(NCORES))
            gx = psA.tile([128, 8], F32, tag="gx")
            for t in range(4):
                nc.tensor.matmul(gx[:, 2 * t:2 * t + 2], lhsT=selT_sb[:],
                                 rhs=gp_sb[:, 2 * t:2 * t + 2], start=True, stop=True)
            gxs = pools.tile([128, 8], F32, tag="gxs")
            nc.vector.tensor_copy(gxs[:], gx[:])
            gx3 = gxs.rearrange("p (t two) -> p t two", two=2)
            musq = pools.tile([128, 4], F32, tag="musq")
            nc.vector.tensor_tensor(out=musq[:], in0=gx3[:, :, 0], in1=gx3[:, :, 0],
                                    op=mybir.AluOpType.mult)
            var = pools.tile([128, 4], F32, tag="var")
            nc.vector.tensor_tensor(out=var[:], in0=gx3[:, :, 1], in1=musq[:],
                                    op=mybir.AluOpType.subtract)
            sd = pools.tile([128, 4], F32, tag="sd")
            nc.scalar.activation(out=sd[:], in_=var[:], func=AF.Sqrt,
                                 bias=eps_sb[:], scale=1.0)
            rstd = pools.tile([128, 4], F32, tag="rstd")
            nc.vector.reciprocal(out=rstd[:], in_=sd[:])
            nc.vector.tensor_tensor(out=A_sb[:], in0=rstd[:], in1=nw_sb[:],
                                    op=mybir.AluOpType.mult)
            muA = pools.tile([128, 4], F32, tag="muA")
            nc.vector.tensor_tensor(out=muA[:], in0=gx3[:, :, 0], in1=A_sb[:],
                                    op=mybir.AluOpType.mult)
            nc.vector.tensor_tensor(out=B_sb[:], in0=nb_sb[:], in1=muA[:],
                                    op=mybir.AluOpType.subtract)

            # ---------- phase C: normalize + SiLU + qkv ----------
            poolq2 = es_cd.enter_context(tc.tile_pool(name="poolq2", bufs=1))
            psB = es_b.enter_context(tc.tile_pool(name="psB", bufs=3, space="PSUM"))

            h_sb = poolq2.tile([128, 2048], F32R)
            for t in range(4):
                nc.scalar.activation(out=h_sb[:, t * 512:(t + 1) * 512],
                                     in_=xblk_sb[:, t * 512:(t + 1) * 512],
                                     func=AF.Silu,
                                     bias=B_sb[:, t:t + 1], scale=A_sb[:, t:t + 1])

            vs = poolq2.tile([128, 2048], BF16)
            # qkv: chunk c of q/k lands at rows 64*(c%2), cols (c//2)*512,
            # i.e. psum halves drain straight into the parity layout
            for ot in range(12):
                ps = psB.tile([128, 512], F32, tag="qkvps")
                for kt in range(4):
                    nc.tensor.matmul(
                        ps[:],
                        lhsT=qkvw_sb[:, kt * 1536 + ot * 128:
                                     kt * 1536 + (ot + 1) * 128],
                        rhs=h_sb[:, kt * 512:(kt + 1) * 512],
                        start=(kt == 0), stop=(kt == 3))
                kind, t = ot // 4, ot % 4
                if kind == 2:
                    nc.vector.tensor_scalar_add(vs[:, t * 512:(t + 1) * 512], ps[:],
                                                qb_sb[:, ot:ot + 1])
                else:
                    dst = QT if kind == 0 else KT
                    nc.vector.tensor_scalar_add(
                        dst[0:64, (2 * t) * 512:(2 * t + 1) * 512],
                        ps[0:64, :], qb_sb[0:64, ot:ot + 1])
                    nc.vector.tensor_scalar_add(
                        dst[64:128, (2 * t + 1) * 512:(2 * t + 2) * 512],
                        ps[64:128, :], qb_sb[64:128, ot:ot + 1])
            # mirror DMAs: copy even chunks (rows 0:64) up, odd chunks down,
            # so both partition halves hold every chunk (for S row tiling)
            for dst in (QT, KT):
                d3 = dst.rearrange("p (u two n) -> p u two n", two=2, n=512)
                nc.sync.dma_start(out=d3[64:128, :, 0, :], in_=d3[0:64, :, 0, :])
                nc.sync.dma_start(out=d3[0:64, :, 1, :], in_=d3[64:128, :, 1, :])

            # ---------- phase D: Vp layout (PE transposes, bf16) ----------
            Vp3 = Vp.rearrange("p (j c) -> p j c", c=65)
            nc.vector.memset(Vp3[:, :, 64:65], 1.0)
            for tt in range(4):
                for b in range(4):
                    pst = psB.tile([128, 128], BF16, tag="vtr")
                    nc.tensor.transpose(
                        pst[:], in_=vs[:, tt * 512 + b * 128:tt * 512 + (b + 1) * 128],
                        identity=id_sb[:])
                    j1, j2 = 8 * tt + b, 8 * tt + 4 + b
                    nc.vector.tensor_copy(Vp3[:, j1, 0:64], pst[:, 0:64])
                    nc.vector.tensor_copy(Vp3[:, j2, 0:64], pst[:, 64:128])

        # ---------- phase E: attention (software-pipelined S/exp | O) ----------
        with ExitStack() as es_e:
            psS = es_e.enter_context(tc.tile_pool(name="psS", bufs=2, space="PSUM"))
            psO = es_e.enter_context(tc.tile_pool(name="psO", bufs=2, space="PSUM"))
            poolPB = es_e.enter_context(tc.tile_pool(name="poolPB", bufs=2))
            poolsm = es_e.enter_context(tc.tile_pool(name="poolsm", bufs=3))

            groups = [(j0, min(3, 32 - j0)) for j0 in range(0, 32, 3)]
            PBts = {}
            opss = {}

            def emit_o_mms(I, j0, glen):
                for jj in range(glen):
                    j = j0 + jj
                    nc.tensor.matmul(opss[I][:], lhsT=Vp3[:, j, 0:65],
                                     rhs=PBts[I][:, j * 512:(j + 1) * 512],
                                     start=(j == 0), stop=(j == 31))

            def emit_o_drain(I):
                isl = slice(I * 512, (I + 1) * 512)
                OuS = poolsm.tile([65, 512], F32, tag="OuS")
                nc.vector.tensor_copy(OuS[:], opss[I][:])
                rD = poolsm.tile([1, 512], F32R, tag="rD")
                with nc.allow_low_precision(reason="f32r output is f32 bits"):
                    nc.vector.reciprocal(out=rD[:], in_=OuS[64:65, :])
                dps = psO.tile([64, 512], F32, tag="ops")
                nc.tensor.matmul(dps[:], lhsT=ones64_sb[:],
                                 rhs=rD[:], start=True, stop=True)
                nc.vector.tensor_tensor(out=ONorm[0:64, isl], in0=OuS[0:64, :],
                                        in1=dps[:], op=mybir.AluOpType.mult)
                del PBts[I], opss[I]

            for I in range(9):
                if I < 8:
                    isl = slice(I * 512, (I + 1) * 512)
                    PBts[I] = poolPB.tile([128, 32 * 512], BF16, tag="PBt",
                                          name=f"PBt{I}")
                    opss[I] = psO.tile([65, 512], F32, tag="ops", name=f"ops{I}")
                for (j0, glen) in groups:
                    if I < 8:
                        sp = psS.tile([128, 1536], F32, tag="sp")
                        for jj in range(glen):
                            j = j0 + jj
                            r = slice(64, 128) if (TILE_S and j % 2) else slice(0, 64)
                            nc.tensor.matmul(
                                sp[:, jj * 512:(jj + 1) * 512],
                                lhsT=KT[r, j * 128:(j + 1) * 128],
                                rhs=QT[r, isl],
                                start=True, stop=True)
                        nc.scalar.activation(
                            out=PBts[I][:, j0 * 512:(j0 + glen) * 512],
                            in_=sp[:, 0:glen * 512], func=AF.Exp, scale=SCALE)
                    if I > 0:
                        emit_o_mms(I - 1, j0, glen)
                if I > 0:
                    emit_o_drain(I - 1)

        # ---------- phase F: proj + bias + residual ----------
        with ExitStack() as es_f:
            psP = es_f.enter_context(tc.tile_pool(name="psP", bufs=2, space="PSUM"))
            poolf = es_f.enter_context(tc.tile_pool(name="poolf", bufs=2))
            for ot in range(4):
                pp = psP.tile([128, 512], F32, tag="pp")
                for chunk in range(8):
                    nc.tensor.matmul(
                        pp[:],
                        lhsT=pwT_sb[0:64, chunk * 512 + ot * 128:
                                    chunk * 512 + (ot + 1) * 128],
                        rhs=ONorm[0:64, chunk * 512:(chunk + 1) * 512],
                        start=(chunk == 0), stop=(chunk == 7))
                fin = poolf.tile([128, 512], F32, tag="fin")
                nc.vector.tensor_scalar_add(fin[:], pp[:], pb_sb[:, ot:ot + 1])
                nc.vector.tensor_tensor(out=fin[:], in0=fin[:],
                                        in1=xblk_sb[:, ot * 512:(ot + 1) * 512],
                                        op=mybir.AluOpType.add)
                nc.sync.dma_start(out=out[ot * 128:(ot + 1) * 128, :], in_=fin[:])


def _host_inputs(x, norm_w, norm_b, qkv_w, qkv_b, proj_w, proj_b):
    import ml_dtypes
    x2d = np.ascontiguousarray(np.asarray(x, np.float32).reshape(CH, N))
    qkv_w = np.asarray(qkv_w, np.float32)
    proj_w = np.asarray(proj_w, np.float32)
    common = {
        "xfb": np.ascontiguousarray(x2d.astype(ml_dtypes.bfloat16)),
        "qkvwT": np.ascontiguousarray(qkv_w.T),
        "qb": np.ascontiguousarray(np.asarray(qkv_b, np.float32).reshape(12, 128).T),
        "pwT": np.ascontiguousarray(
            proj_w.T.reshape(8, 64, CH).transpose(1, 0, 2).reshape(64, 8 * CH)),
        "pb": np.ascontiguousarray(np.asarray(proj_b, np.float32).reshape(4, 128).T),
        "nw": np.ascontiguousarray(np.asarray(norm_w, np.float32).reshape(4, 128).T),
        "nbias": np.ascontiguousarray(np.asarray(norm_b, np.float32).reshape(4, 128).T),
        "identb": np.eye(128, dtype=ml_dtypes.bfloat16),
        "ones64": np.ones((1, 64), np.float32),
        "sel8": np.ascontiguousarray(
            (np.arange(128)[:, None] // GS == np.arange(8)[None, :])
            .astype(np.float32) / GS),
        "selT": np.ascontiguousarray(
            (np.arange(128)[None, :] // GS == np.arange(8)[:, None])
            .astype(np.float32) / NCORES),
    }
    in_maps = []
    for h in range(NCORES):
        m = dict(common)
        m["xblk"] = np.ascontiguousarray(x2d[:, h * NB:(h + 1) * NB])
        in_maps.append(m)
    return in_maps


_LAST_RESULT = {}


def kernel(x, norm_w, norm_b, qkv_w, qkv_b, proj_w, proj_b, _trace=False,
           _tmpdir=None):
    nc = _build()
    in_maps = _host_inputs(x, norm_w, norm_b, qkv_w, qkv_b, proj_w, proj_b)
    res = run_bass_kernel_spmd(nc, in_maps, core_ids=list(range(NCORES)),
                               trace=_trace, tmpdir=_tmpdir)
    _LAST_RESULT["res"] = res
    full = np.concatenate([res.results[h]["out"] for h in range(NCORES)], axis=1)
    return full.reshape(1, CH, 64, 64).astype(np.float32)
